# revision 25
# baseline (speedup 1.0000x reference)
"""Correlation network kernel for Trainium2.

corr[b,i,j,k,l] = sum_c A[b,i,j,c] * B[b,k,l,c]

Per batch b this is  A_b (2304x64) @ B_b^T (64x2304) -> 2304x2304.
Sharding: data-parallel over batch B=8 across the 8 NeuronCores; each core
computes one full 2304x2304 correlation matrix.

The harness gate is a global (Frobenius-style) relative error < 2e-2, so
both inputs and the output are bf16 (total fro err ~2.9e-3). The bf16
output halves the dominant HBM write: 21.2 MB -> 10.6 MB per core (~27-32
us DMA stream at the measured 350-405 GB/s per-queue peak).

Compute: K=64 bf16 matmuls with PE-array row packing — even m-tiles use
array rows 0:63, odd m-tiles rows 64:127 (tile_position derives from the
SBUF base partition). Alternating the two groups lets each group's
LDWEIGHTS overlap the other group's matmul, avoiding the ~170 ns
per-matmul weight-load serialization observed on HW. b_hi is duplicated
into both partition halves so each group reads its own copy.

PSUM->SBUF copies (which also convert fp32->bf16) pair up per chunk: DVE
takes one group, ACT the other, alternating. Output row-blocks go out in
m-tile pairs as single strided DMAs, rotated over three queues (Sync &
ACT HWDGE, gpsimd SWDGE); the first pair streams in fine chunks to prime
the write stream, the last pair is split across two queues to shorten the
tail.
"""

import numpy as np
import ml_dtypes

import concourse.bacc as bacc
import concourse.mybir as mybir
import concourse.tile as tile
from concourse.bass_interp import get_hw_module
from concourse.bass_utils import run_bass_kernel_spmd

B, H, W, C = 8, 48, 48, 64
HW = H * W  # 2304
P = 128
M_TILES = HW // P  # 18
M_PAIRS = M_TILES // 2  # 9
N_TILE = 512
FP32 = mybir.dt.float32
BF16 = mybir.dt.bfloat16
BF16_NP = ml_dtypes.bfloat16

CHUNKS = [(0, 1024), (1024, 1024), (2048, 256)]


def _corr_body(tc, out, ath, bth):
    nc = tc.nc
    with (
        tc.tile_pool(name="ops", bufs=1) as op_pool,
        tc.tile_pool(name="ps", bufs=4, space="PSUM") as ps_pool,
        tc.tile_pool(name="outs", bufs=8) as out_pool,
    ):
        # ath: [128, 1152] -- rows 0:64 = a_hi^T for even m-tiles, rows
        # 64:128 = odd m-tiles. bth: [128, 2304] = [b_hi; b_hi].
        ath_t = op_pool.tile([P, HW // 2], BF16)
        bth_t = op_pool.tile([P, HW], BF16)

        # All input loads on the ACT HWDGE ring: they are issued before any
        # copies exist (no head-of-line blocking of the ACT sequencer) and
        # keep the Sync queue 100% dedicated to the output stream. Chunk
        # order = what pair 0 needs first.
        nc.scalar.dma_start(out=ath_t[:, 0:P], in_=ath[:, 0:P])
        nc.scalar.dma_start(out=bth_t[:, 0:1024], in_=bth[:, 0:1024])
        nc.scalar.dma_start(out=bth_t[:, 1024:HW], in_=bth[:, 1024:HW])
        nc.scalar.dma_start(out=ath_t[:, P : HW // 2], in_=ath[:, P : HW // 2])

        for pp in range(M_PAIRS):
            # per-m out tiles; DMA per m-tile as soon as its copies land
            ot_e = out_pool.tile([P, HW], BF16, tag="ot")
            ot_o = out_pool.tile([P, HW], BF16, tag="ot")
            pcol = slice(pp * P, (pp + 1) * P)
            m_e, m_o = 2 * pp, 2 * pp + 1
            for ci, (c0, csz) in enumerate(CHUNKS):
                ps_e = ps_pool.tile([P, 1024], FP32, tag="ps")
                ps_o = ps_pool.tile([P, 1024], FP32, tag="ps")
                for s0 in range(0, csz, N_TILE):
                    ssz = min(N_TILE, csz - s0)
                    cs = slice(c0 + s0, c0 + s0 + ssz)
                    # alternate groups so LDWEIGHTS of one PE half
                    # overlaps the other half's matmul
                    nc.tensor.matmul(
                        ps_e[:, s0 : s0 + ssz],
                        ath_t[0:64, pcol],
                        bth_t[0:64, cs],
                        start=True,
                        stop=True,
                    )
                    nc.tensor.matmul(
                        ps_o[:, s0 : s0 + ssz],
                        ath_t[64:P, pcol],
                        bth_t[64:P, cs],
                        start=True,
                        stop=True,
                    )
                # one copy per group per chunk; DVE/ACT swap groups per
                # chunk to balance their load
                cslice = slice(c0, c0 + csz)
                if ci % 2 == 0:
                    nc.vector.tensor_copy(ot_e[:, cslice], ps_e[:, :csz])
                    nc.scalar.copy(ot_o[:, cslice], ps_o[:, :csz])
                else:
                    nc.scalar.copy(ot_e[:, cslice], ps_e[:, :csz])
                    nc.vector.tensor_copy(ot_o[:, cslice], ps_o[:, :csz])
            if True:
                nc.sync.dma_start(
                    out=out[m_e * P : (m_e + 1) * P, :], in_=ot_e[:, :]
                )
                nc.sync.dma_start(
                    out=out[m_o * P : (m_o + 1) * P, :], in_=ot_o[:, :]
                )


_NC_CACHE = None


def _build():
    global _NC_CACHE
    if _NC_CACHE is None:
        nc = bacc.Bacc(
            "TRN2",
            target_bir_lowering=False,
            debug=False,
            enable_asserts=False,
        )
        ath = nc.dram_tensor("ath", [P, HW // 2], BF16, kind="ExternalInput").ap()
        bth = nc.dram_tensor("bth", [P, HW], BF16, kind="ExternalInput").ap()
        out = nc.dram_tensor("out", [HW, HW], BF16, kind="ExternalOutput").ap()
        with tile.TileContext(nc) as tc:
            _corr_body(tc, out, ath, bth)
        nc.compile()
        nc.m = get_hw_module(nc.m)
        _NC_CACHE = nc
    return _NC_CACHE


def _pack_lhs(xT):
    """[C, HW] -> [128, HW/2]: rows 0:64 even m-tiles, rows 64:128 odd."""
    t = xT.reshape(C, M_PAIRS, 2, P)  # [c, pair, eo, j]
    return np.ascontiguousarray(t.transpose(2, 0, 1, 3).reshape(2 * C, M_PAIRS * P))


def _prep_inputs(feature_A, feature_B):
    in_maps = []
    for i in range(B):
        aT = np.ascontiguousarray(
            feature_A[i].reshape(HW, C).T, dtype=np.float32
        )  # [C, HW]
        bT = np.ascontiguousarray(
            feature_B[i].reshape(HW, C).T, dtype=np.float32
        )
        ah = aT.astype(BF16_NP)
        bh = bT.astype(BF16_NP)
        in_maps.append(
            {
                "ath": _pack_lhs(ah),
                "bth": np.ascontiguousarray(np.concatenate([bh, bh], axis=0)),
            }
        )
    return in_maps


def _run(feature_A, feature_B, trace=False, **kwargs):
    feature_A = np.asarray(feature_A, dtype=np.float32)
    feature_B = np.asarray(feature_B, dtype=np.float32)
    assert feature_A.shape == (B, H, W, C), feature_A.shape
    assert feature_B.shape == (B, H, W, C), feature_B.shape

    nc = _build()
    in_maps = _prep_inputs(feature_A, feature_B)
    res = run_bass_kernel_spmd(nc, in_maps, list(range(B)), trace=trace, **kwargs)
    out = np.stack(
        [res.results[i]["out"].astype(np.float32) for i in range(B)], axis=0
    )
    return out.reshape(B, H, W, H, W), res


def kernel(feature_A, feature_B):
    out, _ = _run(feature_A, feature_B)
    return out


# revision 26
# speedup vs baseline: 1.1824x; 1.1824x over previous
"""Correlation network kernel for Trainium2.

corr[b,i,j,k,l] = sum_c A[b,i,j,c] * B[b,k,l,c]

Per batch b this is  A_b (2304x64) @ B_b^T (64x2304) -> 2304x2304.
Sharding: data-parallel over batch B=8 across the 8 NeuronCores; each core
computes one full 2304x2304 correlation matrix.

The harness gate is a global (Frobenius-style) relative error < 2e-2, so
both inputs and the output are bf16 (total fro err ~2.9e-3). The bf16
output halves the dominant HBM write: 21.2 MB -> 10.6 MB per core (~27-32
us DMA stream at the measured 350-405 GB/s per-queue peak).

Compute: K=64 bf16 matmuls with PE-array row packing — even m-tiles use
array rows 0:63, odd m-tiles rows 64:127 (tile_position derives from the
SBUF base partition). Alternating the two groups lets each group's
LDWEIGHTS overlap the other group's matmul, avoiding the ~170 ns
per-matmul weight-load serialization observed on HW. b_hi is duplicated
into both partition halves so each group reads its own copy.

PSUM->SBUF copies (which also convert fp32->bf16) pair up per chunk: DVE
takes one group, ACT the other, alternating per chunk to balance load.

All output DMAs are per-m-tile [128, 2304] blocks on the Sync HWDGE ring
ONLY. Measured on HW: (1) DMA issues on the ACT ring head-of-line-block
the ACT sequencer's own copies; (2) the gpsimd SWDGE queue moves outputs
at ~140 GB/s with multi-us latency and holds out tiles hostage; (3) a
single HWDGE queue reaches 350-405 GB/s, and multiple queues alternate
rather than add bandwidth; (4) two-block strided pair DMAs ([p,t,n]
descriptors) are much slower than plain per-m blocks. Input loads ride
the ACT ring, issued before any copies exist so nothing blocks.
"""

import numpy as np
import ml_dtypes

import concourse.bacc as bacc
import concourse.mybir as mybir
import concourse.tile as tile
from concourse.bass_interp import get_hw_module
from concourse.bass_utils import run_bass_kernel_spmd

B, H, W, C = 8, 48, 48, 64
HW = H * W  # 2304
P = 128
M_TILES = HW // P  # 18
M_PAIRS = M_TILES // 2  # 9
N_TILE = 512
FP32 = mybir.dt.float32
BF16 = mybir.dt.bfloat16
BF16_NP = ml_dtypes.bfloat16

CHUNKS = [(0, 1024), (1024, 1024), (2048, 256)]


def _corr_body(tc, out, ath, bth):
    nc = tc.nc
    with (
        tc.tile_pool(name="ops", bufs=1) as op_pool,
        tc.tile_pool(name="ps", bufs=4, space="PSUM") as ps_pool,
        tc.tile_pool(name="outs", bufs=8) as out_pool,
    ):
        # ath: [128, 1152] -- rows 0:64 = a_hi^T for even m-tiles, rows
        # 64:128 = odd m-tiles. bth: [128, 2304] = [b_hi; b_hi].
        ath_t = op_pool.tile([P, HW // 2], BF16)
        bth_t = op_pool.tile([P, HW], BF16)

        # All input loads on the ACT HWDGE ring: they are issued before any
        # copies exist (no head-of-line blocking of the ACT sequencer) and
        # keep the Sync queue 100% dedicated to the output stream. Chunk
        # order = what pair 0 needs first.
        nc.scalar.dma_start(out=ath_t[:, 0:P], in_=ath[:, 0:P])
        nc.scalar.dma_start(out=bth_t[:, 0:1024], in_=bth[:, 0:1024])
        nc.scalar.dma_start(out=bth_t[:, 1024:HW], in_=bth[:, 1024:HW])
        nc.scalar.dma_start(out=ath_t[:, P : HW // 2], in_=ath[:, P : HW // 2])

        for pp in range(M_PAIRS):
            # per-m out tiles; DMA per m-tile as soon as its copies land
            ot_e = out_pool.tile([P, HW], BF16, tag="ot")
            ot_o = out_pool.tile([P, HW], BF16, tag="ot")
            pcol = slice(pp * P, (pp + 1) * P)
            m_e, m_o = 2 * pp, 2 * pp + 1
            for ci, (c0, csz) in enumerate(CHUNKS):
                ps_e = ps_pool.tile([P, 1024], FP32, tag="ps")
                ps_o = ps_pool.tile([P, 1024], FP32, tag="ps")
                for s0 in range(0, csz, N_TILE):
                    ssz = min(N_TILE, csz - s0)
                    cs = slice(c0 + s0, c0 + s0 + ssz)
                    # alternate groups so LDWEIGHTS of one PE half
                    # overlaps the other half's matmul
                    nc.tensor.matmul(
                        ps_e[:, s0 : s0 + ssz],
                        ath_t[0:64, pcol],
                        bth_t[0:64, cs],
                        start=True,
                        stop=True,
                    )
                    nc.tensor.matmul(
                        ps_o[:, s0 : s0 + ssz],
                        ath_t[64:P, pcol],
                        bth_t[64:P, cs],
                        start=True,
                        stop=True,
                    )
                # one copy per group per chunk; DVE/ACT swap groups per
                # chunk to balance their load
                cslice = slice(c0, c0 + csz)
                if ci % 2 == 0:
                    nc.vector.tensor_copy(ot_e[:, cslice], ps_e[:, :csz])
                    nc.scalar.copy(ot_o[:, cslice], ps_o[:, :csz])
                else:
                    nc.scalar.copy(ot_e[:, cslice], ps_e[:, :csz])
                    nc.vector.tensor_copy(ot_o[:, cslice], ps_o[:, :csz])
            if True:
                nc.sync.dma_start(
                    out=out[m_e * P : (m_e + 1) * P, :], in_=ot_e[:, :]
                )
                nc.sync.dma_start(
                    out=out[m_o * P : (m_o + 1) * P, :], in_=ot_o[:, :]
                )


_NC_CACHE = None


def _build():
    global _NC_CACHE
    if _NC_CACHE is None:
        nc = bacc.Bacc(
            "TRN2",
            target_bir_lowering=False,
            debug=False,
            enable_asserts=False,
        )
        ath = nc.dram_tensor("ath", [P, HW // 2], BF16, kind="ExternalInput").ap()
        bth = nc.dram_tensor("bth", [P, HW], BF16, kind="ExternalInput").ap()
        out = nc.dram_tensor("out", [HW, HW], BF16, kind="ExternalOutput").ap()
        with tile.TileContext(nc) as tc:
            _corr_body(tc, out, ath, bth)
        nc.compile()
        nc.m = get_hw_module(nc.m)
        _NC_CACHE = nc
    return _NC_CACHE


def _pack_lhs(xT):
    """[C, HW] -> [128, HW/2]: rows 0:64 even m-tiles, rows 64:128 odd."""
    t = xT.reshape(C, M_PAIRS, 2, P)  # [c, pair, eo, j]
    return np.ascontiguousarray(t.transpose(2, 0, 1, 3).reshape(2 * C, M_PAIRS * P))


def _prep_inputs(feature_A, feature_B):
    in_maps = []
    for i in range(B):
        aT = np.ascontiguousarray(
            feature_A[i].reshape(HW, C).T, dtype=np.float32
        )  # [C, HW]
        bT = np.ascontiguousarray(
            feature_B[i].reshape(HW, C).T, dtype=np.float32
        )
        ah = aT.astype(BF16_NP)
        bh = bT.astype(BF16_NP)
        in_maps.append(
            {
                "ath": _pack_lhs(ah),
                "bth": np.ascontiguousarray(np.concatenate([bh, bh], axis=0)),
            }
        )
    return in_maps


def _run(feature_A, feature_B, trace=False, **kwargs):
    feature_A = np.asarray(feature_A, dtype=np.float32)
    feature_B = np.asarray(feature_B, dtype=np.float32)
    assert feature_A.shape == (B, H, W, C), feature_A.shape
    assert feature_B.shape == (B, H, W, C), feature_B.shape

    nc = _build()
    in_maps = _prep_inputs(feature_A, feature_B)
    res = run_bass_kernel_spmd(nc, in_maps, list(range(B)), trace=trace, **kwargs)
    out = np.stack(
        [res.results[i]["out"].astype(np.float32) for i in range(B)], axis=0
    )
    return out.reshape(B, H, W, H, W), res


def kernel(feature_A, feature_B):
    out, _ = _run(feature_A, feature_B)
    return out
